# revision 32
# baseline (speedup 1.0000x reference)
"""Trainium2 Bass kernel for out = x @ expm(skew(angles)) + bias.

Strategy:
  - Data-parallel over the batch: x [16384, 512] is split into 8 shards of
    [2048, 512], one per NeuronCore. angles/bias are replicated.
  - Host only does layout: builds A = skew(angles), A+I, the fp32r
    rounding of A, and ships each core its x shard pre-transposed ([512, 2048])
    so the contraction dim lands on SBUF partitions (pure marshaling, no
    FLOPs; the PE's matmul contracts over the partition dim, so x^T layout
    is required by the ISA). All linear algebra runs on-device.
  - On each core the rotation is computed via a degree-6 Taylor series in
    Paterson-Stockmeyer form (3 matmuls of 512^3), exploiting skew-symmetry
    so no on-chip transposes of the 512x512 operands are ever needed:
        A2  = A @ A        (lhsT = -A,  since A^T = -A)
        A3n = -A^3         (lhsT = A2,  since A2 is symmetric)
        B'  = A + A2/5 - A3n/30
        F'  = A3 @ B'      (lhsT = A3n, since A3^T = -A3)
        W   = expm(A) = (I + A) + A2/2 - A3n/6 + F'/24
    (the identity is folded in via a host-sent A+I for the linear term).
  - expm matmul operands are float32r (fp32 rounded to 11 mantissa bits):
    the PE streams fp32r at 1 column/cycle vs 4 cycle-equivalents for plain
    fp32's two-pass LOW_HIGH mode.  Operand magnitudes there are ~1e-2, so
    the 2^-12 input rounding contributes only ~3e-5 absolute to the output.
    The main x@W matmul stays full fp32; the linear terms of W are built
    from the exact (unrounded) A.
  - Main loop: per 128-row tile of x, 4 accumulating fp32 matmuls of N=512
    straight from the preloaded x^T slices; the final DVE op adds bias
    while moving PSUM -> SBUF.
  - -A is produced on-device by a DVE negate of the rounded A (exact, and
    fp32r rounding commutes with negation), halving the DMA bytes the expm
    chain waits on at startup.

Truncation error of the degree-6 series for this operand norm
(||A||_2 ~ 0.44) is ~4e-8, below fp32 matmul roundoff.
"""

import numpy as np

import concourse.bacc as bacc
import concourse.bass as bass
import concourse.mybir as mybir
import concourse.tile as tile
from concourse.bass_utils import run_bass_kernel_spmd

DIM = 512
BATCH = 16384
N_CORES = 8
XB = BATCH // N_CORES          # rows per core
P = 128                        # partitions
KT = DIM // P                  # 4 k-tiles
MT = XB // P                   # 16 m-tiles per core
XC = 4                         # m-tiles per x DMA chunk
F32 = mybir.dt.float32
F32R = mybir.dt.float32r

_CACHE = {}


def build_bass():
    nc = bacc.Bacc("TRN2", target_bir_lowering=False, debug=False)

    xt_d = nc.dram_tensor("xt", [DIM, XB], F32, kind="ExternalInput")
    a_d = nc.dram_tensor("a", [DIM, DIM], F32, kind="ExternalInput")
    ai_d = nc.dram_tensor("ai", [DIM, DIM], F32, kind="ExternalInput")
    ar_d = nc.dram_tensor("ar", [DIM, DIM], F32R, kind="ExternalInput")
    biasr_d = nc.dram_tensor("biasr", [P, DIM], F32, kind="ExternalInput")
    out_d = nc.dram_tensor("out", [XB, DIM], F32, kind="ExternalOutput")

    AOP = mybir.AluOpType

    with tile.TileContext(nc) as tc:
        with (
            tc.tile_pool(name="const", bufs=1) as cpool,
            tc.tile_pool(name="xin", bufs=MT // XC) as xpool,
            tc.tile_pool(name="oout", bufs=4) as opool,
            tc.tile_pool(name="eps", bufs=2, space=bass.MemorySpace.PSUM) as eps,
            tc.tile_pool(name="ops", bufs=2, space=bass.MemorySpace.PSUM) as ops,
        ):
            a_sb = cpool.tile([P, KT, DIM], F32)     # [p, t, n] = A[128t+p, n]
            ai_sb = cpool.tile([P, KT, DIM], F32)    # A + I
            ar_sb = cpool.tile([P, KT, DIM], F32R)
            nar_sb = cpool.tile([P, KT, DIM], F32R)
            biasr_sb = cpool.tile([P, DIM], F32)

            for t in range(KT):
                nc.sync.dma_start(ar_sb[:, t, :], ar_d[P * t : P * (t + 1), :])
            # -A is negated on-device (exact; fp32r rounding commutes with
            # negation) instead of being a second 1MB load on the critical path
            for t in range(KT):
                nc.vector.tensor_scalar_mul(nar_sb[:, t, :], ar_sb[:, t, :], -1.0)
            # late-needed inputs, gated behind the expm-critical loads
            nc.sync.dma_start(
                a_sb[:, :, :], a_d[:, :].rearrange("(t p) n -> p t n", p=P)
            )
            nc.sync.dma_start(
                ai_sb[:, :, :], ai_d[:, :].rearrange("(t p) n -> p t n", p=P)
            )
            nc.sync.dma_start(biasr_sb[:, :], biasr_d[:, :])

            # ---- x^T loads: 4 chunks of [512, 512] ----
            xch = []
            for c in range(MT // XC):
                xc = xpool.tile([P, KT, P * XC], F32, tag="x")
                nc.sync.dma_start(
                    xc[:, :, :],
                    xt_d[:, P * XC * c : P * XC * (c + 1)].rearrange(
                        "(t p) m -> p t m", p=P
                    ),
                )
                xch.append(xc)

            # ---- expm chain (replicated; fp32r operands) ----
            a2_sb = cpool.tile([P, KT, DIM], F32R)
            a3n_sb = cpool.tile([P, KT, DIM], F32R)
            bp_sb = cpool.tile([P, KT, DIM], F32R)
            t3_sb = cpool.tile([P, KT, DIM], F32)
            m_sb = cpool.tile([P, KT, DIM], F32)

            # A2 = A @ A
            for i in range(KT):
                ps = eps.tile([P, DIM], F32, tag="eps")
                for t in range(KT):
                    nc.tensor.matmul(
                        ps[:, :],
                        nar_sb[:, t, P * i : P * (i + 1)],
                        ar_sb[:, t, :],
                        start=(t == 0),
                        stop=(t == KT - 1),
                    )
                nc.scalar.copy(a2_sb[:, i, :], ps[:, :])

            # A3n = -(A2 @ A) = A2 @ (-A)
            for i in range(KT):
                ps = eps.tile([P, DIM], F32, tag="eps")
                for t in range(KT):
                    nc.tensor.matmul(
                        ps[:, :],
                        a2_sb[:, t, P * i : P * (i + 1)],
                        nar_sb[:, t, :],
                        start=(t == 0),
                        stop=(t == KT - 1),
                    )
                nc.scalar.copy(a3n_sb[:, i, :], ps[:, :])

            # B' = A + A2/5 - A3n/30 ; t3 = (A + I) + A2/2 - A3n/6
            # (split per k-tile so the F' matmuls can start on bp tile 0
            # while later tiles are still being built)
            # a2-only halves first: they are ready during the A3n matmul
            # phase, so the DVE works ahead and only one op separates the
            # last A3n copy from bp[0] being ready for F'
            for t in range(KT):
                nc.vector.scalar_tensor_tensor(
                    bp_sb[:, t, :], a2_sb[:, t, :], 0.2, a_sb[:, t, :],
                    AOP.mult, AOP.add,
                )
            for t in range(KT):
                nc.vector.scalar_tensor_tensor(
                    bp_sb[:, t, :], a3n_sb[:, t, :], -1.0 / 30.0, bp_sb[:, t, :],
                    AOP.mult, AOP.add,
                )
            for t in range(KT):
                nc.vector.scalar_tensor_tensor(
                    t3_sb[:, t, :], a2_sb[:, t, :], 0.5, ai_sb[:, t, :],
                    AOP.mult, AOP.add,
                )
            for t in range(KT):
                nc.vector.scalar_tensor_tensor(
                    t3_sb[:, t, :], a3n_sb[:, t, :], -1.0 / 6.0, t3_sb[:, t, :],
                    AOP.mult, AOP.add,
                )

            # F' = A3 @ B' ; W = F'/24 + t3
            for i in range(KT):
                ps = eps.tile([P, DIM], F32, tag="eps")
                for t in range(KT):
                    nc.tensor.matmul(
                        ps[:, :],
                        a3n_sb[:, t, P * i : P * (i + 1)],
                        bp_sb[:, t, :],
                        start=(t == 0),
                        stop=(t == KT - 1),
                    )
                nc.vector.scalar_tensor_tensor(
                    m_sb[:, i, :], ps[:, :], 1.0 / 24.0, t3_sb[:, i, :],
                    AOP.mult, AOP.add,
                )

            # ---- main loop: out = x @ W + bias ----
            for mi in range(MT):
                xc = xch[mi // XC]
                mo = P * (mi % XC)
                ps = ops.tile([P, DIM], F32, tag="out")
                for kb in range(KT):
                    nc.tensor.matmul(
                        ps[:, :],
                        xc[:, kb, mo : mo + P],
                        m_sb[:, kb, :],
                        start=(kb == 0),
                        stop=(kb == KT - 1),
                    )
                ot = opool.tile([P, DIM], F32, tag="o")
                nc.vector.tensor_add(ot[:, :], ps[:, :], biasr_sb[:, :])
                nc.sync.dma_start(out_d[P * mi : P * (mi + 1), :], ot[:, :])

    nc.compile()
    return nc


def _get_nc():
    if "nc" not in _CACHE:
        _CACHE["nc"] = build_bass()
    return _CACHE["nc"]


def _round_fp32r(x):
    """Round-to-nearest-even to 11 mantissa bits (verified bit-exact
    against walrus's fp32_to_fp32r)."""
    b = np.ascontiguousarray(x, dtype=np.float32).view(np.uint32).astype(np.uint64)
    b = b + 0x7FF + ((b >> 12) & 1)
    return (b & np.uint64(0xFFFFF000)).astype(np.uint32).view(np.float32)


def _host_inputs(angles, bias):
    angles = np.asarray(angles, dtype=np.float32)
    bias = np.asarray(bias, dtype=np.float32)
    iu, ju = np.triu_indices(DIM, k=1)
    A = np.zeros((DIM, DIM), dtype=np.float32)
    A[iu, ju] = angles
    A[ju, iu] = -angles
    return {
        "a": A,
        "ai": A + np.eye(DIM, dtype=np.float32),
        "ar": _round_fp32r(A),
        "biasr": np.ascontiguousarray(
            np.broadcast_to(bias.reshape(1, DIM), (P, DIM))
        ),
    }


def kernel(x, angles, bias, _profile=False):
    x = np.asarray(x, dtype=np.float32)
    # per-core x shards, pre-transposed to [DIM, XB] (layout only)
    xts = np.ascontiguousarray(
        x.reshape(N_CORES, XB, DIM).transpose(0, 2, 1)
    )
    shared = _host_inputs(angles, bias)
    nc = _get_nc()
    in_maps = [{"xt": xts[c], **shared} for c in range(N_CORES)]
    res = run_bass_kernel_spmd(
        nc, in_maps, list(range(N_CORES)), trace=bool(_profile)
    )
    _CACHE["last_result"] = res
    out = np.concatenate([res.results[c]["out"] for c in range(N_CORES)], axis=0)
    return out


# revision 33
# speedup vs baseline: 1.0047x; 1.0047x over previous
"""Trainium2 Bass kernel for out = x @ expm(skew(angles)) + bias.

Strategy:
  - Data-parallel over the batch: x [16384, 512] is split into 8 shards of
    [2048, 512], one per NeuronCore. angles/bias are replicated.
  - Host only does layout: builds A = skew(angles), A+I, the fp32r
    rounding of A, and ships each core its x shard pre-transposed ([512, 2048])
    so the contraction dim lands on SBUF partitions (pure marshaling, no
    FLOPs; the PE's matmul contracts over the partition dim, so x^T layout
    is required by the ISA). All linear algebra runs on-device.
  - On each core the rotation is computed via a degree-6 Taylor series in
    Paterson-Stockmeyer form (3 matmuls of 512^3), exploiting skew-symmetry
    so no on-chip transposes of the 512x512 operands are ever needed:
        A2  = A @ A        (lhsT = -A,  since A^T = -A)
        A3n = -A^3         (lhsT = A2,  since A2 is symmetric)
        B'  = A + A2/5 - A3n/30
        F'  = A3 @ B'      (lhsT = A3n, since A3^T = -A3)
        W   = expm(A) = (I + A) + A2/2 - A3n/6 + F'/24
    (the identity is folded in via a host-sent A+I for the linear term).
  - expm matmul operands are float32r (fp32 rounded to 11 mantissa bits):
    the PE streams fp32r at 1 column/cycle vs 4 cycle-equivalents for plain
    fp32's two-pass LOW_HIGH mode.  Operand magnitudes there are ~1e-2, so
    the 2^-12 input rounding contributes only ~3e-5 absolute to the output.
    The main x@W matmul stays full fp32; the linear terms of W are built
    from the exact (unrounded) A.
  - Main loop: per 128-row tile of x, 4 accumulating fp32 matmuls of N=512
    straight from the preloaded x^T slices; the final DVE op adds bias
    while moving PSUM -> SBUF.
  - -A is produced on-device by a DVE negate of the rounded A (exact, and
    fp32r rounding commutes with negation), halving the DMA bytes the expm
    chain waits on at startup.

Truncation error of the degree-6 series for this operand norm
(||A||_2 ~ 0.44) is ~4e-8, below fp32 matmul roundoff.
"""

import numpy as np

import concourse.bacc as bacc
import concourse.bass as bass
import concourse.mybir as mybir
import concourse.tile as tile
from concourse.bass_utils import run_bass_kernel_spmd

DIM = 512
BATCH = 16384
N_CORES = 8
XB = BATCH // N_CORES          # rows per core
P = 128                        # partitions
KT = DIM // P                  # 4 k-tiles
MT = XB // P                   # 16 m-tiles per core
XC = 4                         # m-tiles per x DMA chunk
F32 = mybir.dt.float32
F32R = mybir.dt.float32r

_CACHE = {}


def build_bass():
    nc = bacc.Bacc("TRN2", target_bir_lowering=False, debug=False)

    xt_d = nc.dram_tensor("xt", [DIM, XB], F32, kind="ExternalInput")
    a_d = nc.dram_tensor("a", [DIM, DIM], F32, kind="ExternalInput")
    ai_d = nc.dram_tensor("ai", [DIM, DIM], F32, kind="ExternalInput")
    ar_d = nc.dram_tensor("ar", [DIM, DIM], F32R, kind="ExternalInput")
    biasr_d = nc.dram_tensor("biasr", [P, DIM], F32, kind="ExternalInput")
    out_d = nc.dram_tensor("out", [XB, DIM], F32, kind="ExternalOutput")

    AOP = mybir.AluOpType

    with tile.TileContext(nc) as tc:
        with (
            tc.tile_pool(name="const", bufs=1) as cpool,
            tc.tile_pool(name="xin", bufs=MT // XC) as xpool,
            tc.tile_pool(name="oout", bufs=4) as opool,
            tc.tile_pool(name="eps", bufs=2, space=bass.MemorySpace.PSUM) as eps,
            tc.tile_pool(name="ops", bufs=2, space=bass.MemorySpace.PSUM) as ops,
        ):
            a_sb = cpool.tile([P, KT, DIM], F32)     # [p, t, n] = A[128t+p, n]
            ai_sb = cpool.tile([P, KT, DIM], F32)    # A + I
            ar_sb = cpool.tile([P, KT, DIM], F32R)
            nar_sb = cpool.tile([P, KT, DIM], F32R)
            biasr_sb = cpool.tile([P, DIM], F32)

            for t in range(KT):
                nc.sync.dma_start(ar_sb[:, t, :], ar_d[P * t : P * (t + 1), :])
            # -A is negated on-device (exact; fp32r rounding commutes with
            # negation) instead of being a second 1MB load on the critical path
            for t in range(KT):
                nc.vector.tensor_scalar_mul(nar_sb[:, t, :], ar_sb[:, t, :], -1.0)
            # inputs below are consumed well after the expm chain starts
            nc.sync.dma_start(
                a_sb[:, :, :], a_d[:, :].rearrange("(t p) n -> p t n", p=P)
            )
            nc.sync.dma_start(
                ai_sb[:, :, :], ai_d[:, :].rearrange("(t p) n -> p t n", p=P)
            )
            nc.sync.dma_start(biasr_sb[:, :], biasr_d[:, :])

            # ---- x^T loads: 4 chunks of [512, 512] ----
            xch = []
            for c in range(MT // XC):
                xc = xpool.tile([P, KT, P * XC], F32, tag="x")
                nc.sync.dma_start(
                    xc[:, :, :],
                    xt_d[:, P * XC * c : P * XC * (c + 1)].rearrange(
                        "(t p) m -> p t m", p=P
                    ),
                )
                xch.append(xc)

            # ---- expm chain (replicated; fp32r operands) ----
            a2_sb = cpool.tile([P, KT, DIM], F32R)
            a3n_sb = cpool.tile([P, KT, DIM], F32R)
            bp_sb = cpool.tile([P, KT, DIM], F32R)
            t3_sb = cpool.tile([P, KT, DIM], F32)
            m_sb = cpool.tile([P, KT, DIM], F32)

            # A2 = A @ A
            for i in range(KT):
                ps = eps.tile([P, DIM], F32, tag="eps")
                for t in range(KT):
                    nc.tensor.matmul(
                        ps[:, :],
                        nar_sb[:, t, P * i : P * (i + 1)],
                        ar_sb[:, t, :],
                        start=(t == 0),
                        stop=(t == KT - 1),
                    )
                nc.scalar.copy(a2_sb[:, i, :], ps[:, :])

            # A3n = -(A2 @ A) = A2 @ (-A)
            for i in range(KT):
                ps = eps.tile([P, DIM], F32, tag="eps")
                for t in range(KT):
                    nc.tensor.matmul(
                        ps[:, :],
                        a2_sb[:, t, P * i : P * (i + 1)],
                        nar_sb[:, t, :],
                        start=(t == 0),
                        stop=(t == KT - 1),
                    )
                nc.scalar.copy(a3n_sb[:, i, :], ps[:, :])

            # B' = A + A2/5 - A3n/30 ; t3 = (A + I) + A2/2 - A3n/6
            # (split per k-tile so the F' matmuls can start on bp tile 0
            # while later tiles are still being built)
            # a2-only halves first: they are ready during the A3n matmul
            # phase, so the DVE works ahead and only one op separates the
            # last A3n copy from bp[0] being ready for F'
            for t in range(KT):
                nc.vector.scalar_tensor_tensor(
                    bp_sb[:, t, :], a2_sb[:, t, :], 0.2, a_sb[:, t, :],
                    AOP.mult, AOP.add,
                )
            for t in range(KT):
                nc.vector.scalar_tensor_tensor(
                    bp_sb[:, t, :], a3n_sb[:, t, :], -1.0 / 30.0, bp_sb[:, t, :],
                    AOP.mult, AOP.add,
                )
            for t in range(KT):
                nc.vector.scalar_tensor_tensor(
                    t3_sb[:, t, :], a2_sb[:, t, :], 0.5, ai_sb[:, t, :],
                    AOP.mult, AOP.add,
                )
            for t in range(KT):
                nc.vector.scalar_tensor_tensor(
                    t3_sb[:, t, :], a3n_sb[:, t, :], -1.0 / 6.0, t3_sb[:, t, :],
                    AOP.mult, AOP.add,
                )

            # F' = A3 @ B' ; W = F'/24 + t3
            for i in range(KT):
                ps = eps.tile([P, DIM], F32, tag="eps")
                for t in range(KT):
                    nc.tensor.matmul(
                        ps[:, :],
                        a3n_sb[:, t, P * i : P * (i + 1)],
                        bp_sb[:, t, :],
                        start=(t == 0),
                        stop=(t == KT - 1),
                    )
                nc.vector.scalar_tensor_tensor(
                    m_sb[:, i, :], ps[:, :], 1.0 / 24.0, t3_sb[:, i, :],
                    AOP.mult, AOP.add,
                )

            # ---- main loop: out = x @ W + bias ----
            for mi in range(MT):
                xc = xch[mi // XC]
                mo = P * (mi % XC)
                ps = ops.tile([P, DIM], F32, tag="out")
                for kb in range(KT):
                    nc.tensor.matmul(
                        ps[:, :],
                        xc[:, kb, mo : mo + P],
                        m_sb[:, kb, :],
                        start=(kb == 0),
                        stop=(kb == KT - 1),
                    )
                ot = opool.tile([P, DIM], F32, tag="o")
                nc.vector.tensor_add(ot[:, :], ps[:, :], biasr_sb[:, :])
                nc.sync.dma_start(out_d[P * mi : P * (mi + 1), :], ot[:, :])

    nc.compile()
    return nc


def _get_nc():
    if "nc" not in _CACHE:
        _CACHE["nc"] = build_bass()
    return _CACHE["nc"]


def _round_fp32r(x):
    """Round-to-nearest-even to 11 mantissa bits (verified bit-exact
    against walrus's fp32_to_fp32r)."""
    b = np.ascontiguousarray(x, dtype=np.float32).view(np.uint32).astype(np.uint64)
    b = b + 0x7FF + ((b >> 12) & 1)
    return (b & np.uint64(0xFFFFF000)).astype(np.uint32).view(np.float32)


def _host_inputs(angles, bias):
    angles = np.asarray(angles, dtype=np.float32)
    bias = np.asarray(bias, dtype=np.float32)
    iu, ju = np.triu_indices(DIM, k=1)
    A = np.zeros((DIM, DIM), dtype=np.float32)
    A[iu, ju] = angles
    A[ju, iu] = -angles
    return {
        "a": A,
        "ai": A + np.eye(DIM, dtype=np.float32),
        "ar": _round_fp32r(A),
        "biasr": np.ascontiguousarray(
            np.broadcast_to(bias.reshape(1, DIM), (P, DIM))
        ),
    }


def kernel(x, angles, bias, _profile=False):
    x = np.asarray(x, dtype=np.float32)
    # per-core x shards, pre-transposed to [DIM, XB] (layout only)
    xts = np.ascontiguousarray(
        x.reshape(N_CORES, XB, DIM).transpose(0, 2, 1)
    )
    shared = _host_inputs(angles, bias)
    nc = _get_nc()
    in_maps = [{"xt": xts[c], **shared} for c in range(N_CORES)]
    res = run_bass_kernel_spmd(
        nc, in_maps, list(range(N_CORES)), trace=bool(_profile)
    )
    _CACHE["last_result"] = res
    out = np.concatenate([res.results[c]["out"] for c in range(N_CORES)], axis=0)
    return out


# revision 36
# speedup vs baseline: 1.0265x; 1.0216x over previous
"""Trainium2 Bass kernel for out = x @ expm(skew(angles)) + bias.

Strategy:
  - Data-parallel over the batch: x [16384, 512] is split into 8 shards of
    [2048, 512], one per NeuronCore. angles/bias are replicated.
  - Host only does layout: builds A = skew(angles), A+I, the fp32r
    rounding of A, and ships each core its x shard pre-transposed ([512, 2048])
    so the contraction dim lands on SBUF partitions (pure marshaling, no
    FLOPs; the PE's matmul contracts over the partition dim, so x^T layout
    is required by the ISA). All linear algebra runs on-device.
  - On each core the rotation is computed via a degree-6 Taylor series in
    Paterson-Stockmeyer form (3 matmuls of 512^3), exploiting skew-symmetry
    so no on-chip transposes of the 512x512 operands are ever needed:
        A2  = A @ A        (lhsT = -A,  since A^T = -A)
        A3n = -A^3         (lhsT = A2,  since A2 is symmetric)
        B'  = A + A2/5 - A3n/30
        F'  = A3 @ B'      (lhsT = A3n, since A3^T = -A3)
        W   = expm(A) = (I + A) + A2/2 - A3n/6 + F'/24
    (the identity is folded in via a host-sent A+I for the linear term).
  - expm matmul operands are float32r (fp32 rounded to 11 mantissa bits):
    the PE streams fp32r at 1 column/cycle vs 4 cycle-equivalents for plain
    fp32's two-pass LOW_HIGH mode.  Operand magnitudes there are ~1e-2, so
    the 2^-12 input rounding contributes only ~3e-5 absolute to the output.
    The main x@W matmul stays full fp32; the linear terms of W are built
    from the exact (unrounded) A.
  - Main loop: per 128-row tile of x, 4 accumulating fp32 matmuls of N=512
    straight from the preloaded x^T slices; the final DVE op adds bias
    while moving PSUM -> SBUF.
  - -A is produced on-device by a DVE negate of the rounded A (exact, and
    fp32r rounding commutes with negation), halving the DMA bytes the expm
    chain waits on at startup.

Truncation error of the degree-6 series for this operand norm
(||A||_2 ~ 0.44) is ~4e-8, below fp32 matmul roundoff.
"""

import numpy as np

import concourse.bacc as bacc
import concourse.bass as bass
import concourse.mybir as mybir
import concourse.tile as tile
from concourse.bass_utils import run_bass_kernel_spmd

DIM = 512
BATCH = 16384
N_CORES = 8
XB = BATCH // N_CORES          # rows per core
P = 128                        # partitions
KT = DIM // P                  # 4 k-tiles
MT = XB // P                   # 16 m-tiles per core
XC = 4                         # m-tiles per x DMA chunk
F32 = mybir.dt.float32
F32R = mybir.dt.float32r

_CACHE = {}


def build_bass():
    nc = bacc.Bacc("TRN2", target_bir_lowering=False, debug=False)

    xt_d = nc.dram_tensor("xt", [DIM, XB], F32, kind="ExternalInput")
    a_d = nc.dram_tensor("a", [DIM, DIM], F32, kind="ExternalInput")
    ai_d = nc.dram_tensor("ai", [DIM, DIM], F32, kind="ExternalInput")
    ar_d = nc.dram_tensor("ar", [DIM, DIM], F32R, kind="ExternalInput")
    biasr_d = nc.dram_tensor("biasr", [P, DIM], F32, kind="ExternalInput")
    out_d = nc.dram_tensor("out", [XB, DIM], F32, kind="ExternalOutput")

    AOP = mybir.AluOpType

    with tile.TileContext(nc) as tc:
        with (
            tc.tile_pool(name="const", bufs=1) as cpool,
            tc.tile_pool(name="xin", bufs=MT // XC) as xpool,
            tc.tile_pool(name="oout", bufs=4) as opool,
            tc.tile_pool(name="eps", bufs=6, space=bass.MemorySpace.PSUM) as eps,
            tc.tile_pool(name="ops", bufs=2, space=bass.MemorySpace.PSUM) as ops,
        ):
            a_sb = cpool.tile([P, KT, DIM], F32)     # [p, t, n] = A[128t+p, n]
            ai_sb = cpool.tile([P, KT, DIM], F32)    # A + I
            ar_sb = cpool.tile([P, KT, DIM], F32R)
            nar_sb = cpool.tile([P, KT, DIM], F32R)
            biasr_sb = cpool.tile([P, DIM], F32)

            for t in range(KT):
                nc.sync.dma_start(ar_sb[:, t, :], ar_d[P * t : P * (t + 1), :])
            # -A is negated on-device (exact; fp32r rounding commutes with
            # negation) instead of being a second 1MB load on the critical path
            for t in range(KT):
                nc.vector.tensor_scalar_mul(nar_sb[:, t, :], ar_sb[:, t, :], -1.0)
            # inputs below are consumed well after the expm chain starts
            nc.sync.dma_start(
                a_sb[:, :, :], a_d[:, :].rearrange("(t p) n -> p t n", p=P)
            )
            nc.sync.dma_start(
                ai_sb[:, :, :], ai_d[:, :].rearrange("(t p) n -> p t n", p=P)
            )
            nc.sync.dma_start(biasr_sb[:, :], biasr_d[:, :])

            # ---- x^T loads: 4 chunks of [512, 512] ----
            xch = []
            for c in range(MT // XC):
                xc = xpool.tile([P, KT, P * XC], F32, tag="x")
                nc.sync.dma_start(
                    xc[:, :, :],
                    xt_d[:, P * XC * c : P * XC * (c + 1)].rearrange(
                        "(t p) m -> p t m", p=P
                    ),
                )
                xch.append(xc)

            # ---- expm chain (replicated; fp32r operands) ----
            a2_sb = cpool.tile([P, KT, DIM], F32R)
            a3n_sb = cpool.tile([P, KT, DIM], F32R)
            bp_sb = cpool.tile([P, KT, DIM], F32R)
            t3_sb = cpool.tile([P, KT, DIM], F32)
            m_sb = cpool.tile([P, KT, DIM], F32)

            # A2 = A @ A
            for i in range(KT):
                ps = eps.tile([P, DIM], F32, tag="eps")
                for t in range(KT):
                    nc.tensor.matmul(
                        ps[:, :],
                        nar_sb[:, t, P * i : P * (i + 1)],
                        ar_sb[:, t, :],
                        start=(t == 0),
                        stop=(t == KT - 1),
                    )
                nc.scalar.copy(a2_sb[:, i, :], ps[:, :])

            # a2-only halves of B' and t3 go first: the DVE chews through
            # them during the A3n matmul phase, so after the last A3n copy
            # only one op separates bp[0] from being ready for F'
            for t in range(KT):
                nc.vector.scalar_tensor_tensor(
                    bp_sb[:, t, :], a2_sb[:, t, :], 0.2, a_sb[:, t, :],
                    AOP.mult, AOP.add,
                )
            for t in range(KT):
                nc.vector.scalar_tensor_tensor(
                    t3_sb[:, t, :], a2_sb[:, t, :], 0.5, ai_sb[:, t, :],
                    AOP.mult, AOP.add,
                )
            # A3n = -(A2 @ A) = A2 @ (-A)
            for i in range(KT):
                ps = eps.tile([P, DIM], F32, tag="eps")
                for t in range(KT):
                    nc.tensor.matmul(
                        ps[:, :],
                        a2_sb[:, t, P * i : P * (i + 1)],
                        nar_sb[:, t, :],
                        start=(t == 0),
                        stop=(t == KT - 1),
                    )
                nc.scalar.copy(a3n_sb[:, i, :], ps[:, :])

            # B' = A + A2/5 - A3n/30 ; t3 = (A + I) + A2/2 - A3n/6
            # (split per k-tile so the F' matmuls can start on bp tile 0
            # while later tiles are still being built)
            # a2-only halves first: they are ready during the A3n matmul
            # phase, so the DVE works ahead and only one op separates the
            # last A3n copy from bp[0] being ready for F'
            for t in range(KT):
                nc.vector.scalar_tensor_tensor(
                    bp_sb[:, t, :], a3n_sb[:, t, :], -1.0 / 30.0, bp_sb[:, t, :],
                    AOP.mult, AOP.add,
                )
            for t in range(KT):
                nc.vector.scalar_tensor_tensor(
                    t3_sb[:, t, :], a3n_sb[:, t, :], -1.0 / 6.0, t3_sb[:, t, :],
                    AOP.mult, AOP.add,
                )

            # F' = A3 @ B' ; W = F'/24 + t3
            for i in range(KT):
                ps = eps.tile([P, DIM], F32, tag="eps")
                for t in range(KT):
                    nc.tensor.matmul(
                        ps[:, :],
                        a3n_sb[:, t, P * i : P * (i + 1)],
                        bp_sb[:, t, :],
                        start=(t == 0),
                        stop=(t == KT - 1),
                    )
                nc.vector.scalar_tensor_tensor(
                    m_sb[:, i, :], ps[:, :], 1.0 / 24.0, t3_sb[:, i, :],
                    AOP.mult, AOP.add,
                )

            # ---- main loop: out = x @ W + bias ----
            for mi in range(MT):
                xc = xch[mi // XC]
                mo = P * (mi % XC)
                ps = ops.tile([P, DIM], F32, tag="out")
                for kb in range(KT):
                    nc.tensor.matmul(
                        ps[:, :],
                        xc[:, kb, mo : mo + P],
                        m_sb[:, kb, :],
                        start=(kb == 0),
                        stop=(kb == KT - 1),
                    )
                ot = opool.tile([P, DIM], F32, tag="o")
                nc.vector.tensor_add(ot[:, :], ps[:, :], biasr_sb[:, :])
                nc.sync.dma_start(out_d[P * mi : P * (mi + 1), :], ot[:, :])

    nc.compile()
    return nc


def _get_nc():
    if "nc" not in _CACHE:
        _CACHE["nc"] = build_bass()
    return _CACHE["nc"]


def _round_fp32r(x):
    """Round-to-nearest-even to 11 mantissa bits (verified bit-exact
    against walrus's fp32_to_fp32r)."""
    b = np.ascontiguousarray(x, dtype=np.float32).view(np.uint32).astype(np.uint64)
    b = b + 0x7FF + ((b >> 12) & 1)
    return (b & np.uint64(0xFFFFF000)).astype(np.uint32).view(np.float32)


def _host_inputs(angles, bias):
    angles = np.asarray(angles, dtype=np.float32)
    bias = np.asarray(bias, dtype=np.float32)
    iu, ju = np.triu_indices(DIM, k=1)
    A = np.zeros((DIM, DIM), dtype=np.float32)
    A[iu, ju] = angles
    A[ju, iu] = -angles
    return {
        "a": A,
        "ai": A + np.eye(DIM, dtype=np.float32),
        "ar": _round_fp32r(A),
        "biasr": np.ascontiguousarray(
            np.broadcast_to(bias.reshape(1, DIM), (P, DIM))
        ),
    }


def kernel(x, angles, bias, _profile=False):
    x = np.asarray(x, dtype=np.float32)
    # per-core x shards, pre-transposed to [DIM, XB] (layout only)
    xts = np.ascontiguousarray(
        x.reshape(N_CORES, XB, DIM).transpose(0, 2, 1)
    )
    shared = _host_inputs(angles, bias)
    nc = _get_nc()
    in_maps = [{"xt": xts[c], **shared} for c in range(N_CORES)]
    res = run_bass_kernel_spmd(
        nc, in_maps, list(range(N_CORES)), trace=bool(_profile)
    )
    _CACHE["last_result"] = res
    out = np.concatenate([res.results[c]["out"] for c in range(N_CORES)], axis=0)
    return out


# revision 37
# speedup vs baseline: 1.0686x; 1.0410x over previous
"""Trainium2 Bass kernel for out = x @ expm(skew(angles)) + bias.

Strategy:
  - Data-parallel over the batch: x [16384, 512] is split into 8 shards of
    [2048, 512], one per NeuronCore. angles/bias are replicated.
  - Host only does layout: builds A = skew(angles), A+I, the fp32r
    rounding of A, and ships each core its x shard pre-transposed ([512, 2048])
    so the contraction dim lands on SBUF partitions (pure marshaling, no
    FLOPs; the PE's matmul contracts over the partition dim, so x^T layout
    is required by the ISA). All linear algebra runs on-device.
  - On each core the rotation is computed via a degree-6 Taylor series in
    Paterson-Stockmeyer form (3 matmuls of 512^3), exploiting skew-symmetry
    so no on-chip transposes of the 512x512 operands are ever needed:
        A2  = A @ A        (lhsT = -A,  since A^T = -A)
        A3n = -A^3         (lhsT = A2,  since A2 is symmetric)
        B'' = (A+I) + A2/5 - A3n/30
        F'' = A3 @ B''     (lhsT = A3n, since A3^T = -A3)
        W   = expm(A) = (I + A) + A2/2 - A3n/8 + F''/24
    (identical to the degree-6 series: F'' = A3@B' - A3n, and the shifted
    A3n coefficient compensates; only the host-sent A+I is ever needed).
  - expm matmul operands are float32r (fp32 rounded to 11 mantissa bits):
    the PE streams fp32r at 1 column/cycle vs 4 cycle-equivalents for plain
    fp32's two-pass LOW_HIGH mode.  Operand magnitudes there are ~1e-2, so
    the 2^-12 input rounding contributes only ~3e-5 absolute to the output.
    The main x@W matmul stays full fp32; the linear terms of W are built
    from the exact (unrounded) A.
  - Main loop: per 128-row tile of x, 4 accumulating fp32 matmuls of N=512
    straight from the preloaded x^T slices; the final DVE op adds bias
    while moving PSUM -> SBUF.
  - -A is produced on-device by a DVE negate of the rounded A (exact, and
    fp32r rounding commutes with negation), halving the DMA bytes the expm
    chain waits on at startup.

Truncation error of the degree-6 series for this operand norm
(||A||_2 ~ 0.44) is ~4e-8, below fp32 matmul roundoff.
"""

import numpy as np

import concourse.bacc as bacc
import concourse.bass as bass
import concourse.mybir as mybir
import concourse.tile as tile
from concourse.bass_utils import run_bass_kernel_spmd

DIM = 512
BATCH = 16384
N_CORES = 8
XB = BATCH // N_CORES          # rows per core
P = 128                        # partitions
KT = DIM // P                  # 4 k-tiles
MT = XB // P                   # 16 m-tiles per core
XC = 4                         # m-tiles per x DMA chunk
F32 = mybir.dt.float32
F32R = mybir.dt.float32r

_CACHE = {}


def build_bass():
    nc = bacc.Bacc("TRN2", target_bir_lowering=False, debug=False)

    xt_d = nc.dram_tensor("xt", [DIM, XB], F32, kind="ExternalInput")
    ai_d = nc.dram_tensor("ai", [DIM, DIM], F32, kind="ExternalInput")
    ar_d = nc.dram_tensor("ar", [DIM, DIM], F32R, kind="ExternalInput")
    biasr_d = nc.dram_tensor("biasr", [P, DIM], F32, kind="ExternalInput")
    out_d = nc.dram_tensor("out", [XB, DIM], F32, kind="ExternalOutput")

    AOP = mybir.AluOpType

    with tile.TileContext(nc) as tc:
        with (
            tc.tile_pool(name="const", bufs=1) as cpool,
            tc.tile_pool(name="xin", bufs=MT // XC) as xpool,
            tc.tile_pool(name="oout", bufs=4) as opool,
            tc.tile_pool(name="eps", bufs=6, space=bass.MemorySpace.PSUM) as eps,
            tc.tile_pool(name="ops", bufs=2, space=bass.MemorySpace.PSUM) as ops,
        ):
            ai_sb = cpool.tile([P, KT, DIM], F32)    # A + I
            ar_sb = cpool.tile([P, KT, DIM], F32R)
            nar_sb = cpool.tile([P, KT, DIM], F32R)
            biasr_sb = cpool.tile([P, DIM], F32)

            for t in range(KT):
                nc.sync.dma_start(ar_sb[:, t, :], ar_d[P * t : P * (t + 1), :])
            # -A is negated on-device (exact; fp32r rounding commutes with
            # negation) instead of being a second 1MB load on the critical path
            for t in range(KT):
                nc.vector.tensor_scalar_mul(nar_sb[:, t, :], ar_sb[:, t, :], -1.0)
            # inputs below are consumed well after the expm chain starts
            nc.sync.dma_start(
                ai_sb[:, :, :], ai_d[:, :].rearrange("(t p) n -> p t n", p=P)
            )
            nc.sync.dma_start(biasr_sb[:, :], biasr_d[:, :])

            # ---- x^T loads: 4 chunks of [512, 512] ----
            xch = []
            for c in range(MT // XC):
                xc = xpool.tile([P, KT, P * XC], F32, tag="x")
                nc.sync.dma_start(
                    xc[:, :, :],
                    xt_d[:, P * XC * c : P * XC * (c + 1)].rearrange(
                        "(t p) m -> p t m", p=P
                    ),
                )
                xch.append(xc)

            # ---- expm chain (replicated; fp32r operands) ----
            a2_sb = cpool.tile([P, KT, DIM], F32R)
            a3n_sb = cpool.tile([P, KT, DIM], F32R)
            bp_sb = cpool.tile([P, KT, DIM], F32R)
            t3_sb = cpool.tile([P, KT, DIM], F32)
            m_sb = cpool.tile([P, KT, DIM], F32)

            # A2 = A @ A
            for i in range(KT):
                ps = eps.tile([P, DIM], F32, tag="eps")
                for t in range(KT):
                    nc.tensor.matmul(
                        ps[:, :],
                        nar_sb[:, t, P * i : P * (i + 1)],
                        ar_sb[:, t, :],
                        start=(t == 0),
                        stop=(t == KT - 1),
                    )
                nc.scalar.copy(a2_sb[:, i, :], ps[:, :])

            # a2-only halves of B' and t3 go first: the DVE chews through
            # them during the A3n matmul phase, so after the last A3n copy
            # only one op separates bp[0] from being ready for F'
            for t in range(KT):
                nc.vector.scalar_tensor_tensor(
                    bp_sb[:, t, :], a2_sb[:, t, :], 0.2, ai_sb[:, t, :],
                    AOP.mult, AOP.add,
                )
            for t in range(KT):
                nc.vector.scalar_tensor_tensor(
                    t3_sb[:, t, :], a2_sb[:, t, :], 0.5, ai_sb[:, t, :],
                    AOP.mult, AOP.add,
                )
            # A3n = -(A2 @ A) = A2 @ (-A)
            for i in range(KT):
                ps = eps.tile([P, DIM], F32, tag="eps")
                for t in range(KT):
                    nc.tensor.matmul(
                        ps[:, :],
                        a2_sb[:, t, P * i : P * (i + 1)],
                        nar_sb[:, t, :],
                        start=(t == 0),
                        stop=(t == KT - 1),
                    )
                nc.scalar.copy(a3n_sb[:, i, :], ps[:, :])

            # B' = A + A2/5 - A3n/30 ; t3 = (A + I) + A2/2 - A3n/6
            # (split per k-tile so the F' matmuls can start on bp tile 0
            # while later tiles are still being built)
            # a2-only halves first: they are ready during the A3n matmul
            # phase, so the DVE works ahead and only one op separates the
            # last A3n copy from bp[0] being ready for F'
            for t in range(KT):
                nc.vector.scalar_tensor_tensor(
                    bp_sb[:, t, :], a3n_sb[:, t, :], -1.0 / 30.0, bp_sb[:, t, :],
                    AOP.mult, AOP.add,
                )
            for t in range(KT):
                nc.vector.scalar_tensor_tensor(
                    t3_sb[:, t, :], a3n_sb[:, t, :], -1.0 / 8.0, t3_sb[:, t, :],
                    AOP.mult, AOP.add,
                )

            # F' = A3 @ B' ; W = F'/24 + t3
            for i in range(KT):
                ps = eps.tile([P, DIM], F32, tag="eps")
                for t in range(KT):
                    nc.tensor.matmul(
                        ps[:, :],
                        a3n_sb[:, t, P * i : P * (i + 1)],
                        bp_sb[:, t, :],
                        start=(t == 0),
                        stop=(t == KT - 1),
                    )
                nc.vector.scalar_tensor_tensor(
                    m_sb[:, i, :], ps[:, :], 1.0 / 24.0, t3_sb[:, i, :],
                    AOP.mult, AOP.add,
                )

            # ---- main loop: out = x @ W + bias ----
            for mi in range(MT):
                xc = xch[mi // XC]
                mo = P * (mi % XC)
                ps = ops.tile([P, DIM], F32, tag="out")
                for kb in range(KT):
                    nc.tensor.matmul(
                        ps[:, :],
                        xc[:, kb, mo : mo + P],
                        m_sb[:, kb, :],
                        start=(kb == 0),
                        stop=(kb == KT - 1),
                    )
                ot = opool.tile([P, DIM], F32, tag="o")
                nc.vector.tensor_add(ot[:, :], ps[:, :], biasr_sb[:, :])
                nc.sync.dma_start(out_d[P * mi : P * (mi + 1), :], ot[:, :])

    nc.compile()
    return nc


def _get_nc():
    if "nc" not in _CACHE:
        _CACHE["nc"] = build_bass()
    return _CACHE["nc"]


def _round_fp32r(x):
    """Round-to-nearest-even to 11 mantissa bits (verified bit-exact
    against walrus's fp32_to_fp32r)."""
    b = np.ascontiguousarray(x, dtype=np.float32).view(np.uint32).astype(np.uint64)
    b = b + 0x7FF + ((b >> 12) & 1)
    return (b & np.uint64(0xFFFFF000)).astype(np.uint32).view(np.float32)


def _host_inputs(angles, bias):
    angles = np.asarray(angles, dtype=np.float32)
    bias = np.asarray(bias, dtype=np.float32)
    iu, ju = np.triu_indices(DIM, k=1)
    A = np.zeros((DIM, DIM), dtype=np.float32)
    A[iu, ju] = angles
    A[ju, iu] = -angles
    return {
        "ai": A + np.eye(DIM, dtype=np.float32),
        "ar": _round_fp32r(A),
        "biasr": np.ascontiguousarray(
            np.broadcast_to(bias.reshape(1, DIM), (P, DIM))
        ),
    }


def kernel(x, angles, bias, _profile=False):
    x = np.asarray(x, dtype=np.float32)
    # per-core x shards, pre-transposed to [DIM, XB] (layout only)
    xts = np.ascontiguousarray(
        x.reshape(N_CORES, XB, DIM).transpose(0, 2, 1)
    )
    shared = _host_inputs(angles, bias)
    nc = _get_nc()
    in_maps = [{"xt": xts[c], **shared} for c in range(N_CORES)]
    res = run_bass_kernel_spmd(
        nc, in_maps, list(range(N_CORES)), trace=bool(_profile)
    )
    _CACHE["last_result"] = res
    out = np.concatenate([res.results[c]["out"] for c in range(N_CORES)], axis=0)
    return out
